# revision 37
# baseline (speedup 1.0000x reference)
"""MoE layer (E=8 experts, top-2, D=1024, H=4096, N=4096 tokens) on 8 TRN2
NeuronCores.

Strategy: expert-parallel with host-side routed dispatch (the all-to-all of
the sharding hint, performed during input sharding). The host computes the
gate (softmax + top-2, ~0.05% of total FLOPs), gathers each expert's routed
tokens, and ships core e exactly expert e's token batch padded to a static
capacity C (max routed tokens over experts, rounded up to 128). Each core
then runs only its own expert's FFN in transposed activation layout
(feature dim on partitions, tokens on the free axis):

    hT[h, t]   = gelu(sum_d w1[d, h] * xT[d, t] + b1[h])
    outT[d, t] = s[t] * (sum_h w2[h, d] * hT[h, t] + b2[d])

where s[t] is the token's renormalized top-2 gate for this expert. The host
adds the two expert contributions per token when unsharding. This computes
only the routed 2/8 of the dense-expert FLOPs (the baseline computed all 8
experts per token).

Matmuls run in bf16 (1 PE cycle/row, same rate as fp32r but half the DMA
and SBUF) with fp32 PSUM accumulation; bf16 rounding error ~0.5% is well
inside the 2e-2 gate. Tokens are processed in PSUM-bank-sized chunks of
<=512 along the free axis.

Empirically-settled constraints (from trace bisects on this HW):
- capacity C MUST be a multiple of 128: a 72-row tail chunk slowed every
  matmul in the kernel by ~21% (walrus splits non-aligned frees);
- weight tiles must be streamed via 2+ full-partition DMA calls each
  (per-call transfer rate is limited; partition-sliced DMAs are slower
  still), with a pooled double-buffer;
- Tile dependencies are whole-tile, so weights use per-h/per-d pool tiles
  rather than one big resident tensor.
"""

import numpy as np
import ml_dtypes

import concourse.bass as bass  # noqa: F401  (bass types used via tile/bacc)
import concourse.mybir as mybir
import concourse.tile as tile
from concourse import bacc, bass_utils

F32 = mybir.dt.float32
BF16 = mybir.dt.bfloat16
AFT = mybir.ActivationFunctionType
ALU = mybir.AluOpType
NPBF16 = np.dtype(ml_dtypes.bfloat16)

E = 8          # experts
D = 1024       # model dim
H = 4096       # expert hidden dim
P = 128        # partitions
NCORES = 8
NTOK = 4096    # total tokens (B*T = 2*2048)
KD = D // P    # 8 contraction chunks of D
NH = H // P    # 32 h tiles
ND = D // P    # 8 d tiles

_NC = {}       # compiled modules keyed by capacity C
_WCACHE = {}   # host-side weight prep cache keyed by id(w1)
_ACT = AFT.Gelu  # swapped to Tanh by the CoreSim debug harness (no Gelu there)


def _chunks(C):
    out, off = [], 0
    while off < C:
        sz = min(512, C - off)
        out.append((off, sz))
        off += sz
    return out


def _build(C):
    chunks = _chunks(C)
    nc = bacc.Bacc("TRN2", target_bir_lowering=False, debug=False,
                   num_devices=NCORES)
    xg = nc.dram_tensor("xg", [P, KD, C], BF16, kind="ExternalInput").ap()
    w1t = nc.dram_tensor("w1t", [NH, P, KD, P], BF16,
                         kind="ExternalInput").ap()
    b1t = nc.dram_tensor("b1t", [P, NH], F32, kind="ExternalInput").ap()
    w2t = nc.dram_tensor("w2t", [ND, P, NH, P], BF16,
                         kind="ExternalInput").ap()
    b2t = nc.dram_tensor("b2t", [P, ND], F32, kind="ExternalInput").ap()
    sv = nc.dram_tensor("sv", [1, C], F32, kind="ExternalInput").ap()
    outT = nc.dram_tensor("outT", [ND, P, C], BF16,
                          kind="ExternalOutput").ap()

    with tile.TileContext(nc) as tc:
        with (
            tc.tile_pool(name="const", bufs=1) as cpool,
            tc.tile_pool(name="w2p", bufs=2) as w2p,
            tc.tile_pool(name="res", bufs=4) as resp,
            tc.tile_pool(name="psp", bufs=8, space="PSUM") as psp,
        ):
            # ---- persistent SBUF. xg is split into four tiles (kd halves
            # x column ranges): mm1 runs chunk-major, so the first psum
            # group needs only the first 512 token-columns (1MB) instead
            # of the whole 2.4MB. Tile deps are whole-tile, hence separate
            # tiles; startup DMAs ride three engine queues in parallel. ----
            CB = min(512, C)
            xga0 = cpool.tile([P, KD // 2, CB], BF16)
            xgb0 = cpool.tile([P, KD // 2, CB], BF16)
            for kd in range(KD // 2):
                nc.scalar.dma_start(xga0[:, kd, :], xg[:, kd, 0:CB])
            for kd in range(KD // 2):
                nc.gpsimd.dma_start(xgb0[:, kd, :],
                                    xg[:, KD // 2 + kd, 0:CB])
            # the remaining token columns are loaded later (triggers are
            # emitted after a few c0 gelus on the scalar queue) so the
            # first chunk's 1MB gets the full early DMA bandwidth; c1
            # isn't consumed until ~70us in, leaving ample slack
            CR = C - CB
            if CR:
                xga1 = cpool.tile([P, KD // 2, CR], BF16)
                xgb1 = cpool.tile([P, KD // 2, CR], BF16)

            def xg_ap(kd, off, sz):
                t0, t1 = (xga0, xga1) if kd < KD // 2 else (xgb0, xgb1)
                k = kd % (KD // 2)
                if off < CB:
                    return t0[:, k, off:off + sz]
                return t1[:, k, off - CB:off - CB + sz]

            b1_s = cpool.tile([P, NH], F32)
            nc.sync.dma_start(b1_s[:], b1t[:])
            b2_s = cpool.tile([P, ND], F32)
            nc.sync.dma_start(b2_s[:], b2t[:])
            s1 = cpool.tile([1, C], F32)
            nc.sync.dma_start(s1[:], sv[:])
            sb = cpool.tile([P, C], F32)
            nc.gpsimd.partition_broadcast(sb[:], s1[:])
            gT = cpool.tile([P, NH, C], BF16)   # gelu outputs

            # ---- mm1: hT = gelu(w1.T @ xT + b1), chunk-major so the
            # first 54us of PE work needs only the first token chunk.
            # w1 stays resident in individual tiles (reused per chunk). ----
            # only the first 4 w1 tiles load at t=0; the rest are
            # launched just-in-time from the scalar queue (staggered
            # behind the c0 gelus) so they don't steal early DMA
            # bandwidth from the critical-path xg chunk
            w1_tiles = []
            for h in range(NH):
                w1_s = cpool.tile([P, KD, P], BF16, name=f"w1s{h}")
                w1_tiles.append(w1_s)
                if h < 4:
                    nc.sync.dma_start(w1_s[:, :KD // 2, :],
                                      w1t[h, :, :KD // 2, :])
                    nc.sync.dma_start(w1_s[:, KD // 2:, :],
                                      w1t[h, :, KD // 2:, :])
            for ci, (off, sz) in enumerate(chunks):
                for h in range(NH):
                    w1_s = w1_tiles[h]
                    ph = psp.tile([P, sz], F32, tag="ps")
                    for kd in range(KD):
                        nc.tensor.matmul(ph[:], w1_s[:, kd, :],
                                         xg_ap(kd, off, sz),
                                         start=(kd == 0), stop=(kd == KD - 1))
                    nc.scalar.activation(gT[:, h, off:off + sz], ph[:],
                                         _ACT, bias=b1_s[:, h:h + 1])
                    if ci == 0 and h + 4 < NH:
                        nxt = w1_tiles[h + 4]
                        nc.scalar.dma_start(nxt[:, :KD // 2, :],
                                            w1t[h + 4, :, :KD // 2, :])
                        nc.scalar.dma_start(nxt[:, KD // 2:, :],
                                            w1t[h + 4, :, KD // 2:, :])
                    if ci == 0 and h == 3 and CR:
                        for kd in range(KD // 2):
                            nc.scalar.dma_start(xga1[:, kd, :],
                                                xg[:, kd, CB:])
                        for kd in range(KD // 2):
                            nc.scalar.dma_start(xgb1[:, kd, :],
                                                xg[:, KD // 2 + kd, CB:])

            # ---- mm2: outT = s * (w2.T @ hT + b2) ----
            for d in range(ND):
                w2_s = w2p.tile([P, NH, P], BF16)
                for q in range(4):
                    nc.sync.dma_start(
                        w2_s[:, q * NH // 4:(q + 1) * NH // 4, :],
                        w2t[d, :, q * NH // 4:(q + 1) * NH // 4, :])
                for off, sz in chunks:
                    po = psp.tile([P, sz], F32, tag="ps")
                    for hh in range(NH):
                        nc.tensor.matmul(po[:], w2_s[:, hh, :],
                                         gT[:, hh, off:off + sz],
                                         start=(hh == 0), stop=(hh == NH - 1))
                    ot = resp.tile([P, sz], F32)
                    nc.vector.tensor_scalar(ot[:], po[:], b2_s[:, d:d + 1],
                                            None, op0=ALU.add)
                    ob = resp.tile([P, sz], BF16, tag="ob")
                    nc.vector.tensor_mul(ob[:], ot[:], sb[:, off:off + sz])
                    nc.sync.dma_start(outT[d, :, off:off + sz], ob[:])

    nc.compile()
    return nc


def _get_nc(C):
    if C not in _NC:
        _NC[C] = _build(C)
    return _NC[C]


def _route(xf, gate_w, gate_b):
    """Replicates the reference gate: softmax, top-2, renormalize."""
    logits = xf @ gate_w + gate_b
    m = logits.max(-1, keepdims=True)
    ex = np.exp(logits - m)
    gates = ex / ex.sum(-1, keepdims=True)
    # stable argsort descending == jax.lax.top_k tie-break (lowest index)
    order = np.argsort(-gates, axis=-1, kind="stable")
    top2 = order[:, :2]
    tg = np.take_along_axis(gates, top2, axis=-1)
    tg = (tg / (tg.sum(-1, keepdims=True) + 1e-9)).astype(np.float32)
    return top2, tg


def _prep_weights(w1, b1, w2, b2):
    key = (id(w1), id(w2))
    if key in _WCACHE:
        return _WCACHE[key]
    f = np.float32
    w1 = np.asarray(w1, f)
    b1 = np.asarray(b1, f)
    w2 = np.asarray(w2, f)
    b2 = np.asarray(b2, f)
    per_core = []
    for e in range(E):
        # w1t[h, p, kd, q] = w1[e, kd*P+p, h*P+q]
        w1t = np.ascontiguousarray(
            w1[e].reshape(KD, P, NH, P).transpose(2, 1, 0, 3)).astype(NPBF16)
        b1t = np.ascontiguousarray(b1[e].reshape(NH, P).T)
        # w2t[d, p, h, q] = w2[e, h*P+p, d*P+q]
        w2t = np.ascontiguousarray(
            w2[e].reshape(NH, P, ND, P).transpose(2, 1, 0, 3)).astype(NPBF16)
        b2t = np.ascontiguousarray(b2[e].reshape(ND, P).T)
        per_core.append({"w1t": w1t, "b1t": b1t, "w2t": w2t, "b2t": b2t})
    _WCACHE.clear()
    _WCACHE[key] = per_core
    return per_core


def _prep(inputs):
    f = np.float32
    x = np.asarray(inputs["x"], f)
    gate_w = np.asarray(inputs["gate_w"], f)
    gate_b = np.asarray(inputs["gate_b"], f)
    xf = x.reshape(NTOK, D)
    top2, tg = _route(xf, gate_w, gate_b)
    wmaps = _prep_weights(inputs["w1"], inputs["b1"],
                          inputs["w2"], inputs["b2"])

    idx_lists = []
    pos = np.zeros((NTOK, E), np.int64)
    for e in range(E):
        mask = (top2[:, 0] == e) | (top2[:, 1] == e)
        idx = np.nonzero(mask)[0]
        pos[idx, e] = np.arange(len(idx))
        idx_lists.append(idx)
    cmax = max(len(i) for i in idx_lists)
    C = max(512, -(-cmax // 128) * 128)

    comb = np.zeros((NTOK, E), f)
    np.put_along_axis(comb, top2, tg, axis=-1)

    in_maps = []
    for e in range(E):
        idx = idx_lists[e]
        c = len(idx)
        xpad = np.zeros((C, D), f)
        xpad[:c] = xf[idx]
        xgv = np.ascontiguousarray(
            xpad.T.reshape(KD, P, C).transpose(1, 0, 2)).astype(NPBF16)
        s = np.zeros((1, C), f)
        s[0, :c] = comb[idx, e]
        m = dict(wmaps[e])
        m["xg"] = xgv
        m["sv"] = s
        in_maps.append(m)
    return in_maps, C, top2, pos, idx_lists


def _assemble(results, C, top2, pos):
    R = np.stack([np.asarray(results[c]["outT"]).astype(np.float32)
                  .reshape(D, C).T for c in range(NCORES)])
    t = np.arange(NTOK)
    out = (R[top2[:, 0], pos[t, top2[:, 0]]]
           + R[top2[:, 1], pos[t, top2[:, 1]]])
    return out.reshape(2, 2048, D).astype(np.float32)


def run(inputs, trace=False, tmpdir=None):
    """Run the kernel; returns (output, exec_time_ns or None)."""
    in_maps, C, top2, pos, _ = _prep(inputs)
    nc = _get_nc(C)
    for attempt in range(2):
        res = bass_utils.run_bass_kernel_spmd(
            nc, in_maps, core_ids=list(range(NCORES)), trace=trace,
            tmpdir=tmpdir)
        out = _assemble(res.results, C, top2, pos)
        if np.isfinite(out).all():
            break
        # transient device glitch (seen once after a crashed profiling
        # session): retry a single time
    return out, res.exec_time_ns


def kernel(**inputs):
    out, _ = run(inputs, trace=False)
    return out


# revision 38
# speedup vs baseline: 1.0204x; 1.0204x over previous
"""MoE layer (E=8 experts, top-2, D=1024, H=4096, N=4096 tokens) on 8 TRN2
NeuronCores.

Strategy: expert-parallel with host-side routed dispatch (the all-to-all of
the sharding hint, performed during input sharding). The host computes the
gate (softmax + top-2, ~0.05% of total FLOPs), gathers each expert's routed
tokens, and ships core e exactly expert e's token batch padded to a static
capacity C (max routed tokens over experts, rounded up to 128). Each core
then runs only its own expert's FFN in transposed activation layout
(feature dim on partitions, tokens on the free axis):

    hT[h, t]   = gelu(sum_d w1[d, h] * xT[d, t] + b1[h])
    outT[d, t] = s[t] * (sum_h w2[h, d] * hT[h, t] + b2[d])

where s[t] is the token's renormalized top-2 gate for this expert. The host
adds the two expert contributions per token when unsharding. This computes
only the routed 2/8 of the dense-expert FLOPs (the baseline computed all 8
experts per token).

Matmuls run in bf16 (1 PE cycle/row, same rate as fp32r but half the DMA
and SBUF) with fp32 PSUM accumulation; bf16 rounding error ~0.5% is well
inside the 2e-2 gate. Tokens are processed in PSUM-bank-sized chunks of
<=512 along the free axis.

Empirically-settled constraints (from trace bisects on this HW):
- capacity C MUST be a multiple of 128: a 72-row tail chunk slowed every
  matmul in the kernel by ~21% (walrus splits non-aligned frees);
- weight tiles must be streamed via 2+ full-partition DMA calls each
  (per-call transfer rate is limited; partition-sliced DMAs are slower
  still), with a pooled double-buffer;
- Tile dependencies are whole-tile, so weights use per-h/per-d pool tiles
  rather than one big resident tensor.
"""

import numpy as np
import ml_dtypes

import concourse.bass as bass  # noqa: F401  (bass types used via tile/bacc)
import concourse.mybir as mybir
import concourse.tile as tile
from concourse import bacc, bass_utils

F32 = mybir.dt.float32
BF16 = mybir.dt.bfloat16
AFT = mybir.ActivationFunctionType
ALU = mybir.AluOpType
NPBF16 = np.dtype(ml_dtypes.bfloat16)

E = 8          # experts
D = 1024       # model dim
H = 4096       # expert hidden dim
P = 128        # partitions
NCORES = 8
NTOK = 4096    # total tokens (B*T = 2*2048)
KD = D // P    # 8 contraction chunks of D
NH = H // P    # 32 h tiles
ND = D // P    # 8 d tiles

_NC = {}       # compiled modules keyed by capacity C
_WCACHE = {}   # host-side weight prep cache keyed by id(w1)
_ACT = AFT.Gelu  # swapped to Tanh by the CoreSim debug harness (no Gelu there)


def _chunks(C):
    out, off = [], 0
    while off < C:
        sz = min(512, C - off)
        out.append((off, sz))
        off += sz
    return out


def _build(C):
    chunks = _chunks(C)
    nc = bacc.Bacc("TRN2", target_bir_lowering=False, debug=False,
                   num_devices=NCORES)
    xg = nc.dram_tensor("xg", [P, KD, C], BF16, kind="ExternalInput").ap()
    w1t = nc.dram_tensor("w1t", [NH, P, KD, P], BF16,
                         kind="ExternalInput").ap()
    b1t = nc.dram_tensor("b1t", [P, NH], F32, kind="ExternalInput").ap()
    w2t = nc.dram_tensor("w2t", [ND, P, NH, P], BF16,
                         kind="ExternalInput").ap()
    b2t = nc.dram_tensor("b2t", [P, ND], F32, kind="ExternalInput").ap()
    sv = nc.dram_tensor("sv", [1, C], F32, kind="ExternalInput").ap()
    outT = nc.dram_tensor("outT", [ND, P, C], BF16,
                          kind="ExternalOutput").ap()

    with tile.TileContext(nc) as tc:
        with (
            tc.tile_pool(name="const", bufs=1) as cpool,
            tc.tile_pool(name="w2p", bufs=2) as w2p,
            tc.tile_pool(name="res", bufs=4) as resp,
            tc.tile_pool(name="psp", bufs=8, space="PSUM") as psp,
        ):
            # ---- persistent SBUF. xg is split into four tiles (kd halves
            # x column ranges): mm1 runs chunk-major, so the first psum
            # group needs only the first 512 token-columns (1MB) instead
            # of the whole 2.4MB. Tile deps are whole-tile, hence separate
            # tiles; startup DMAs ride three engine queues in parallel. ----
            CB = min(512, C)
            xga0 = cpool.tile([P, KD // 2, CB], BF16)
            xgb0 = cpool.tile([P, KD // 2, CB], BF16)
            for kd in range(KD // 2):
                nc.scalar.dma_start(xga0[:, kd, :], xg[:, kd, 0:CB])
            for kd in range(KD // 2):
                nc.gpsimd.dma_start(xgb0[:, kd, :],
                                    xg[:, KD // 2 + kd, 0:CB])
            # the remaining token columns are loaded later (triggers are
            # emitted after a few c0 gelus on the scalar queue) so the
            # first chunk's 1MB gets the full early DMA bandwidth; c1
            # isn't consumed until ~70us in, leaving ample slack
            CR = C - CB
            if CR:
                xga1 = cpool.tile([P, KD // 2, CR], BF16)
                xgb1 = cpool.tile([P, KD // 2, CR], BF16)

            def xg_ap(kd, off, sz):
                t0, t1 = (xga0, xga1) if kd < KD // 2 else (xgb0, xgb1)
                k = kd % (KD // 2)
                if off < CB:
                    return t0[:, k, off:off + sz]
                return t1[:, k, off - CB:off - CB + sz]

            b1_s = cpool.tile([P, NH], F32)
            nc.sync.dma_start(b1_s[:], b1t[:])
            b2_s = cpool.tile([P, ND], F32)
            nc.sync.dma_start(b2_s[:], b2t[:])
            s1 = cpool.tile([1, C], F32)
            nc.sync.dma_start(s1[:], sv[:])
            sb = cpool.tile([P, C], F32)
            nc.gpsimd.partition_broadcast(sb[:], s1[:])
            gT = cpool.tile([P, NH, C], BF16)   # gelu outputs

            # ---- mm1: hT = gelu(w1.T @ xT + b1), chunk-major so the
            # first 54us of PE work needs only the first token chunk.
            # w1 stays resident in individual tiles (reused per chunk). ----
            # only the first 4 w1 tiles load at t=0; the rest are
            # launched just-in-time from the scalar queue (staggered
            # behind the c0 gelus) so they don't steal early DMA
            # bandwidth from the critical-path xg chunk
            w1_tiles = []
            for h in range(NH):
                w1_s = cpool.tile([P, KD, P], BF16, name=f"w1s{h}")
                w1_tiles.append(w1_s)
                nc.sync.dma_start(w1_s[:, :KD // 2, :],
                                  w1t[h, :, :KD // 2, :])
                nc.sync.dma_start(w1_s[:, KD // 2:, :],
                                  w1t[h, :, KD // 2:, :])
            for ci, (off, sz) in enumerate(chunks):
                for h in range(NH):
                    w1_s = w1_tiles[h]
                    ph = psp.tile([P, sz], F32, tag="ps")
                    for kd in range(KD):
                        nc.tensor.matmul(ph[:], w1_s[:, kd, :],
                                         xg_ap(kd, off, sz),
                                         start=(kd == 0), stop=(kd == KD - 1))
                    nc.scalar.activation(gT[:, h, off:off + sz], ph[:],
                                         _ACT, bias=b1_s[:, h:h + 1])
                    if ci == 0 and h == 3 and CR:
                        for kd in range(KD // 2):
                            nc.scalar.dma_start(xga1[:, kd, :],
                                                xg[:, kd, CB:])
                        for kd in range(KD // 2):
                            nc.scalar.dma_start(xgb1[:, kd, :],
                                                xg[:, KD // 2 + kd, CB:])

            # ---- mm2: outT = s * (w2.T @ hT + b2) ----
            for d in range(ND):
                w2_s = w2p.tile([P, NH, P], BF16)
                for q in range(4):
                    nc.sync.dma_start(
                        w2_s[:, q * NH // 4:(q + 1) * NH // 4, :],
                        w2t[d, :, q * NH // 4:(q + 1) * NH // 4, :])
                for off, sz in chunks:
                    po = psp.tile([P, sz], F32, tag="ps")
                    for hh in range(NH):
                        nc.tensor.matmul(po[:], w2_s[:, hh, :],
                                         gT[:, hh, off:off + sz],
                                         start=(hh == 0), stop=(hh == NH - 1))
                    ot = resp.tile([P, sz], F32)
                    nc.vector.tensor_scalar(ot[:], po[:], b2_s[:, d:d + 1],
                                            None, op0=ALU.add)
                    ob = resp.tile([P, sz], BF16, tag="ob")
                    nc.vector.tensor_mul(ob[:], ot[:], sb[:, off:off + sz])
                    nc.sync.dma_start(outT[d, :, off:off + sz], ob[:])

    nc.compile()
    return nc


def _get_nc(C):
    if C not in _NC:
        _NC[C] = _build(C)
    return _NC[C]


def _route(xf, gate_w, gate_b):
    """Replicates the reference gate: softmax, top-2, renormalize."""
    logits = xf @ gate_w + gate_b
    m = logits.max(-1, keepdims=True)
    ex = np.exp(logits - m)
    gates = ex / ex.sum(-1, keepdims=True)
    # stable argsort descending == jax.lax.top_k tie-break (lowest index)
    order = np.argsort(-gates, axis=-1, kind="stable")
    top2 = order[:, :2]
    tg = np.take_along_axis(gates, top2, axis=-1)
    tg = (tg / (tg.sum(-1, keepdims=True) + 1e-9)).astype(np.float32)
    return top2, tg


def _prep_weights(w1, b1, w2, b2):
    key = (id(w1), id(w2))
    if key in _WCACHE:
        return _WCACHE[key]
    f = np.float32
    w1 = np.asarray(w1, f)
    b1 = np.asarray(b1, f)
    w2 = np.asarray(w2, f)
    b2 = np.asarray(b2, f)
    per_core = []
    for e in range(E):
        # w1t[h, p, kd, q] = w1[e, kd*P+p, h*P+q]
        w1t = np.ascontiguousarray(
            w1[e].reshape(KD, P, NH, P).transpose(2, 1, 0, 3)).astype(NPBF16)
        b1t = np.ascontiguousarray(b1[e].reshape(NH, P).T)
        # w2t[d, p, h, q] = w2[e, h*P+p, d*P+q]
        w2t = np.ascontiguousarray(
            w2[e].reshape(NH, P, ND, P).transpose(2, 1, 0, 3)).astype(NPBF16)
        b2t = np.ascontiguousarray(b2[e].reshape(ND, P).T)
        per_core.append({"w1t": w1t, "b1t": b1t, "w2t": w2t, "b2t": b2t})
    _WCACHE.clear()
    _WCACHE[key] = per_core
    return per_core


def _prep(inputs):
    f = np.float32
    x = np.asarray(inputs["x"], f)
    gate_w = np.asarray(inputs["gate_w"], f)
    gate_b = np.asarray(inputs["gate_b"], f)
    xf = x.reshape(NTOK, D)
    top2, tg = _route(xf, gate_w, gate_b)
    wmaps = _prep_weights(inputs["w1"], inputs["b1"],
                          inputs["w2"], inputs["b2"])

    idx_lists = []
    pos = np.zeros((NTOK, E), np.int64)
    for e in range(E):
        mask = (top2[:, 0] == e) | (top2[:, 1] == e)
        idx = np.nonzero(mask)[0]
        pos[idx, e] = np.arange(len(idx))
        idx_lists.append(idx)
    cmax = max(len(i) for i in idx_lists)
    C = max(512, -(-cmax // 128) * 128)

    comb = np.zeros((NTOK, E), f)
    np.put_along_axis(comb, top2, tg, axis=-1)

    in_maps = []
    for e in range(E):
        idx = idx_lists[e]
        c = len(idx)
        xpad = np.zeros((C, D), f)
        xpad[:c] = xf[idx]
        xgv = np.ascontiguousarray(
            xpad.T.reshape(KD, P, C).transpose(1, 0, 2)).astype(NPBF16)
        s = np.zeros((1, C), f)
        s[0, :c] = comb[idx, e]
        m = dict(wmaps[e])
        m["xg"] = xgv
        m["sv"] = s
        in_maps.append(m)
    return in_maps, C, top2, pos, idx_lists


def _assemble(results, C, top2, pos):
    R = np.stack([np.asarray(results[c]["outT"]).astype(np.float32)
                  .reshape(D, C).T for c in range(NCORES)])
    t = np.arange(NTOK)
    out = (R[top2[:, 0], pos[t, top2[:, 0]]]
           + R[top2[:, 1], pos[t, top2[:, 1]]])
    return out.reshape(2, 2048, D).astype(np.float32)


def run(inputs, trace=False, tmpdir=None):
    """Run the kernel; returns (output, exec_time_ns or None)."""
    in_maps, C, top2, pos, _ = _prep(inputs)
    nc = _get_nc(C)
    for attempt in range(2):
        res = bass_utils.run_bass_kernel_spmd(
            nc, in_maps, core_ids=list(range(NCORES)), trace=trace,
            tmpdir=tmpdir)
        out = _assemble(res.results, C, top2, pos)
        if np.isfinite(out).all():
            break
        # transient device glitch (seen once after a crashed profiling
        # session): retry a single time
    return out, res.exec_time_ns


def kernel(**inputs):
    out, _ = run(inputs, trace=False)
    return out
